# revision 20
# baseline (speedup 1.0000x reference)
"""Multi-head attention (B=8, S=1024, E=768, H=12) on 8 trn2 NeuronCores.

Strategy: batch-parallel — core b processes batch element b end-to-end, no
collectives.  All matmuls run in bf16 with fp32 PSUM accumulation.

Per-core dataflow (token index s/t, feature e, head h, head-dim d):
  xT[e, s]   = PE-transpose of x (48 128x128 blocks), cast to bf16
  qT[hd, s]  = WqT_aug^T @ xT_aug   (bias + 1/8 scale folded into weights)
  kT[hd, s]  = WkT_aug^T @ xT_aug
  v[t, hdA]  = xT_aug^T @ WvT_aug   (interleaved [h*65+d] layout, col h*65+64
                                     is all-ones -> gives softmax denominator)
  scoresT[t, s] per head = kT_h^T @ qT_h   (K=64; heads 2j/2j+1 run on
                                            disjoint PE row groups)
  expT = exp(scoresT)  (ACT, PSUM->SBUF bf16; no max-subtraction needed:
                        |scores| < ~6 for this distribution)
  attn_aug[65, s] = v_h_aug^T @ expT  (row 64 = sum_t expT = denominator)
  catT[hd, s] = attn_aug[0:64] * (1/denom)  (DVE recip + DMA partition bcast)
  out[s, f] = catT_aug^T @ WoT_aug   (bias row folded; direct [s,f] layout)
"""

import os
import numpy as np
import ml_dtypes

B, S, E, H, DH = 8, 1024, 768, 12, 64
EA = E + 1          # augmented contraction dim (ones/bias row)
HW = DH + 1         # per-head V width (d cols + ones col)
VW = H * HW         # 780
NT = S // 128       # 8 token tiles
NE = E // 128       # 6 feature tiles

_cache = {}


def _split_multiwaits(nc):
    """This toolchain's walrus encodes at most one sync-wait per instruction
    (two for EventSemaphore).  Tile's epilogue can attach more; hoist the
    extras onto same-engine NOPs placed immediately before the instruction —
    the engine sequencer executes in order, so semantics are unchanged."""
    import concourse.mybir as mybir

    for bb in nc.main_func.blocks:
        out, changed = [], False
        for ins in bb.instructions:
            si = ins.sync_info
            cap = 2 if isinstance(ins, mybir.InstEventSemaphore) else 1
            if si is not None and si.on_wait and len(si.on_wait) > cap:
                waits = list(si.on_wait)
                for w_i, w in enumerate(waits[:-cap]):
                    out.append(mybir.InstNoOp(
                        name=f"{ins.name}-wsplit{w_i}",
                        engine=ins.engine,
                        sync_info=mybir.SyncInfo(on_wait=[w], on_update=[]),
                        bass_nofuse=True,
                    ))
                ins.sync_info = mybir.SyncInfo(
                    on_wait=waits[-cap:], on_update=list(si.on_update))
                changed = True
            out.append(ins)
        if changed:
            bb.instructions = out


def _build_bass(split_waits=True):
    import concourse.bass as bass
    import concourse.tile as tile
    import concourse.mybir as mybir
    from concourse import library_config
    from concourse.masks import make_identity

    f32 = mybir.dt.float32
    bf16 = mybir.dt.bfloat16
    EXP = mybir.ActivationFunctionType.Exp
    LOG = mybir.ActivationFunctionType.Ln

    nc = bass.Bass(trn_type="TRN2")

    x_d = nc.dram_tensor("x", [S, E], f32, kind="ExternalInput")
    wqt_d = nc.dram_tensor("wqt", [EA, E], bf16, kind="ExternalInput")
    wkt_d = nc.dram_tensor("wkt", [EA, E], bf16, kind="ExternalInput")
    wvt_d = nc.dram_tensor("wvt", [EA, VW], bf16, kind="ExternalInput")
    wot_d = nc.dram_tensor("wot", [EA, E], bf16, kind="ExternalInput")
    out_d = nc.dram_tensor("out", [S, E], f32, kind="ExternalOutput")

    from contextlib import ExitStack

    with tile.TileContext(nc) as tc, ExitStack() as ctx:
        singles = ctx.enter_context(tc.tile_pool(name="singles", bufs=1))

        ident = singles.tile([128, 128], f32)
        make_identity(nc, ident)

        ones_row = singles.tile([1, 1024], bf16)
        nc.vector.memset(ones_row, 1.0)

        # ---- weights to SBUF ----
        def load_w(dram, width):
            tiles = []
            for k in range(NE):
                t = singles.tile([128, width], bf16, tag=f"w{dram.name}{k}", name=f"w{dram.name}{k}")
                nc.sync.dma_start(out=t, in_=dram[k * 128:(k + 1) * 128, :])
                tiles.append(t)
            t = singles.tile([1, width], bf16, tag=f"w{dram.name}b", name=f"w{dram.name}b")
            nc.sync.dma_start(out=t, in_=dram[E:EA, :])
            tiles.append(t)
            return tiles

        wq = load_w(wqt_d, E)
        wk = load_w(wkt_d, E)
        wv = load_w(wvt_d, VW)
        wo = load_w(wot_d, E)

        # ---- P1: x -> xT (bf16) ----
        xt = [singles.tile([128, S], bf16, tag=f"xt{j}", name=f"xt{j}")
              for j in range(NE)]

        with tc.tile_pool(name="xload", bufs=8) as xload, \
             tc.tile_pool(name="ps_xt", bufs=4, space="PSUM") as ps_xt:
            xsb = []
            for i in range(NT):
                t = xload.tile([128, E], f32, tag="x", name=f"x{i}")
                nc.sync.dma_start(out=t, in_=x_d[i * 128:(i + 1) * 128, :])
                xsb.append(t)
            for ib in range(2):
                for j in range(NE):
                    ps = ps_xt.tile([128, 512], f32, tag="pxt", name=f"pxt{ib}_{j}")
                    for ii in range(4):
                        nc.tensor.transpose(
                            ps[:, ii * 128:(ii + 1) * 128],
                            xsb[ib * 4 + ii][:, j * 128:(j + 1) * 128],
                            ident,
                        )
                    nc.vector.tensor_copy(
                        xt[j][:, ib * 512:(ib + 1) * 512], ps)

        # ---- P2: projections ----
        qt = [singles.tile([128, S], bf16, tag=f"qt{j}", name=f"qt{j}")
              for j in range(NE)]
        kt = [singles.tile([128, S], bf16, tag=f"kt{j}", name=f"kt{j}")
              for j in range(NE)]
        vt = [singles.tile([128, VW], bf16, tag=f"vt{i}", name=f"vt{i}")
              for i in range(NT)]

        def xa(k):  # augmented xT rows
            return xt[k] if k < NE else ones_row

        with tc.tile_pool(name="ps_proj", bufs=4, space="PSUM") as ps_proj, \
             tc.tile_pool(name="ps_v", bufs=2, space="PSUM") as ps_v:
            for dst, w in ((qt, wq), (kt, wk)):
                for m in range(NE):
                    for sc in range(2):
                        ps = ps_proj.tile([128, 512], f32, tag="pp", name=f"pp{m}_{sc}")
                        for k in range(NE + 1):
                            nc.tensor.matmul(
                                ps,
                                lhsT=w[k][:, m * 128:(m + 1) * 128],
                                rhs=xa(k)[:, sc * 512:(sc + 1) * 512],
                                start=(k == 0), stop=(k == NE),
                            )
                        nc.vector.tensor_copy(
                            dst[m][:, sc * 512:(sc + 1) * 512], ps)
            for i in range(NT):
                ps = ps_v.tile([128, VW], f32, tag="pv", name=f"pv{i}")
                for off, sz in ((0, 512), (512, VW - 512)):
                    for k in range(NE + 1):
                        nc.tensor.matmul(
                            ps[:, off:off + sz],
                            lhsT=xa(k)[:, i * 128:(i + 1) * 128],
                            rhs=wv[k][:, off:off + sz],
                            start=(k == 0), stop=(k == NE),
                        )
                nc.vector.tensor_copy(vt[i], ps)

        # ---- P3: attention ----
        catt = [singles.tile([128, S], bf16, tag=f"ct{j}", name=f"ct{j}")
              for j in range(NE)]

        with tc.tile_pool(name="exp", bufs=18) as expp, \
             tc.tile_pool(name="norm", bufs=4) as normp, \
             tc.tile_pool(name="ps_sc", bufs=2, space="PSUM") as ps_sc, \
             tc.tile_pool(name="ps_at", bufs=2, space="PSUM") as ps_at, \
             tc.tile_pool(name="dscr", bufs=4, space="DRAM") as dscr:
            for hp in range(H // 2):
                exps = [[], []]
                for t in range(NT):
                    for half in range(2):
                        lo, hi = half * 64, half * 64 + 64
                        ps = ps_sc.tile([128, 1024], f32, tag="sc", name=f"sc{hp}_{t}_{half}")
                        for sc in range(2):
                            nc.tensor.matmul(
                                ps[:, sc * 512:(sc + 1) * 512],
                                lhsT=kt[hp][lo:hi, t * 128:(t + 1) * 128],
                                rhs=qt[hp][lo:hi, sc * 512:(sc + 1) * 512],
                                start=True, stop=True,
                                tile_position=(lo, 0),
                            )
                        ex = expp.tile([128, 1024], bf16, tag="e", name=f"e{hp}_{t}_{half}")
                        nc.scalar.activation(ex, ps, EXP)
                        exps[half].append(ex)
                for half in range(2):
                    head = hp * 2 + half
                    pa = ps_at.tile([HW, 1024], f32, tag="at", name=f"at{head}")
                    for t in range(NT):
                        for sc in range(2):
                            nc.tensor.matmul(
                                pa[:, sc * 512:(sc + 1) * 512],
                                lhsT=vt[t][:, head * HW:(head + 1) * HW],
                                rhs=exps[half][t][:, sc * 512:(sc + 1) * 512],
                                start=(t == 0), stop=(t == NT - 1),
                            )
                    lg = normp.tile([1, 1024], f32, tag="lg", name=f"lg{head}")
                    nc.scalar.activation(lg, pa[64:65, :], LOG)
                    recip = normp.tile([1, 1024], f32, tag="r", name=f"r{head}")
                    nc.scalar.activation(recip, lg, EXP, scale=-1.0)
                    dsc = dscr.tile([1, 1024], f32, tag="d", name=f"d{head}")
                    nc.sync.dma_start(out=dsc, in_=recip)
                    recipb = normp.tile([64, 1024], f32, tag="rb", name=f"rb{head}")
                    nc.sync.dma_start(
                        out=recipb, in_=dsc[0].partition_broadcast(64))
                    nc.vector.tensor_mul(
                        catt[hp][half * 64:(half + 1) * 64, :],
                        pa[0:64, :], recipb)

        # ---- P4: output projection ----
        def ca(k):
            return catt[k] if k < NE else ones_row

        with tc.tile_pool(name="osb", bufs=3) as osb, \
             tc.tile_pool(name="ps_o", bufs=2, space="PSUM") as ps_o:
            for m in range(NT):
                ps = ps_o.tile([128, E], f32, tag="po", name=f"po{m}")
                for off, sz in ((0, 512), (512, E - 512)):
                    for k in range(NE + 1):
                        nc.tensor.matmul(
                            ps[:, off:off + sz],
                            lhsT=ca(k)[:, m * 128:(m + 1) * 128],
                            rhs=wo[k][:, off:off + sz],
                            start=(k == 0), stop=(k == NE),
                        )
                ot = osb.tile([128, E], f32, tag="o", name=f"ot{m}")
                nc.vector.tensor_copy(ot, ps)
                nc.sync.dma_start(out=out_d[m * 128:(m + 1) * 128, :], in_=ot)

    if split_waits:
        _split_multiwaits(nc)
    return nc


def _prep_weights(Wq, bq, Wk, bk, Wv, bv, Wo, bo):
    bf16 = ml_dtypes.bfloat16
    scale = 1.0 / np.sqrt(np.float32(DH))

    def aug_qk(W, b, s):
        flat = (W.reshape(H * DH, E) * s).astype(np.float32)     # [hd, e]
        return np.concatenate([flat.T, (b.reshape(1, H * DH) * s)],
                              axis=0).astype(bf16)               # [EA, hd]

    wqt = aug_qk(np.asarray(Wq, np.float32), np.asarray(bq, np.float32), scale)
    wkt = aug_qk(np.asarray(Wk, np.float32), np.asarray(bk, np.float32), 1.0)

    wvt = np.zeros((EA, VW), np.float32)
    Wv = np.asarray(Wv, np.float32)
    bv = np.asarray(bv, np.float32)
    for h in range(H):
        wvt[0:E, h * HW:h * HW + DH] = Wv[h].T
        wvt[E, h * HW:h * HW + DH] = bv[h]
        wvt[E, h * HW + DH] = 1.0
    wvt = wvt.astype(bf16)

    Wo = np.asarray(Wo, np.float32)
    bo = np.asarray(bo, np.float32)
    wot = np.concatenate([Wo.T, bo.reshape(1, E)], axis=0).astype(bf16)
    return wqt, wkt, wvt, wot


def _install_ntff_shim():
    """Provide antenv.axon_hooks (absent in this image) so trace=True can
    drive NRT profiling through libaxon_pjrt.so.  Dev-only; harmless no-op
    when anything is missing."""
    import sys, types
    try:
        import antenv.axon_hooks  # noqa
        return
    except ImportError:
        pass
    try:
        import antenv
        mod = types.ModuleType("antenv.axon_hooks")
        _state = {}
        mod.set_axon_ntff_profile_hook = lambda h: _state.update(h=h)
        mod.get_axon_ntff_profile_hook = lambda: _state.get("h")
        sys.modules["antenv.axon_hooks"] = mod
        antenv.axon_hooks = mod
        from trn_agent_boot.trn_boot import _ntff_profile_via_ctypes
        hook = _ntff_profile_via_ctypes("/opt/axon/libaxon_pjrt.so")
        if hook is not None:
            mod.set_axon_ntff_profile_hook(hook)
    except Exception as e:  # pragma: no cover
        print(f"ntff shim failed: {e}")


def kernel(x, Wq, bq, Wk, bk, Wv, bv, Wo, bo):
    from concourse.bass_utils import run_bass_kernel_spmd

    if "nc" not in _cache:
        _cache["nc"] = _build_bass()
    nc = _cache["nc"]

    wqt, wkt, wvt, wot = _prep_weights(Wq, bq, Wk, bk, Wv, bv, Wo, bo)
    x = np.asarray(x, np.float32)
    in_maps = [
        {"x": np.ascontiguousarray(x[b]),
         "wqt": wqt, "wkt": wkt, "wvt": wvt, "wot": wot}
        for b in range(B)
    ]
    trace = bool(int(os.environ.get("MHA_TRACE", "0")))
    if trace:
        _install_ntff_shim()
    res = run_bass_kernel_spmd(nc, in_maps, list(range(B)), trace=trace)
    _cache["last_results"] = res
    return np.stack([res.results[b]["out"] for b in range(B)]).astype(np.float32)


# revision 24
# speedup vs baseline: 1.0211x; 1.0211x over previous
"""Multi-head attention (B=8, S=1024, E=768, H=12) on 8 trn2 NeuronCores.

Strategy: batch-parallel — core b processes batch element b end-to-end, no
collectives.  All matmuls run in bf16 with fp32 PSUM accumulation.

Per-core dataflow (token index s/t, feature e, head h, head-dim d):
  xT[e, s]   = PE-transpose of x (48 128x128 blocks), cast to bf16
  qT[hd, s]  = WqT_aug^T @ xT_aug   (bias + 1/8 scale folded into weights)
  kT[hd, s]  = WkT_aug^T @ xT_aug
  v[t, hdA]  = xT_aug^T @ WvT_aug   (interleaved [h*65+d] layout, col h*65+64
                                     is all-ones -> gives softmax denominator)
  scoresT[t, s] per head = kT_h^T @ qT_h   (K=64; heads 2j/2j+1 run on
                                            disjoint PE row groups)
  expT = exp(scoresT)  (ACT, PSUM->SBUF bf16; no max-subtraction needed:
                        |scores| < ~6 for this distribution)
  attn_aug[65, s] = v_h_aug^T @ expT  (row 64 = sum_t expT = denominator)
  catT[hd, s] = attn_aug[0:64] * (1/denom)  (DVE recip + DMA partition bcast)
  out[s, f] = catT_aug^T @ WoT_aug   (bias row folded; direct [s,f] layout)
"""

import os
import numpy as np
import ml_dtypes

B, S, E, H, DH = 8, 1024, 768, 12, 64
EA = E + 1          # augmented contraction dim (ones/bias row)
HW = DH + 1         # per-head V width (d cols + ones col)
VW = H * HW         # 780
NT = S // 128       # 8 token tiles
NE = E // 128       # 6 feature tiles

_cache = {}


def _split_multiwaits(nc):
    """This toolchain's walrus encodes at most one sync-wait per instruction
    (two for EventSemaphore).  Tile's epilogue can attach more; hoist the
    extras onto same-engine NOPs placed immediately before the instruction —
    the engine sequencer executes in order, so semantics are unchanged."""
    import concourse.mybir as mybir

    for bb in nc.main_func.blocks:
        out, changed = [], False
        for ins in bb.instructions:
            si = ins.sync_info
            cap = 2 if isinstance(ins, mybir.InstEventSemaphore) else 1
            if si is not None and si.on_wait and len(si.on_wait) > cap:
                waits = list(si.on_wait)
                for w_i, w in enumerate(waits[:-cap]):
                    out.append(mybir.InstNoOp(
                        name=f"{ins.name}-wsplit{w_i}",
                        engine=ins.engine,
                        sync_info=mybir.SyncInfo(on_wait=[w], on_update=[]),
                        bass_nofuse=True,
                    ))
                ins.sync_info = mybir.SyncInfo(
                    on_wait=waits[-cap:], on_update=list(si.on_update))
                changed = True
            out.append(ins)
        if changed:
            bb.instructions = out


def _build_bass(split_waits=True):
    import concourse.bass as bass
    import concourse.tile as tile
    import concourse.mybir as mybir

    from concourse.masks import make_identity

    f32 = mybir.dt.float32
    bf16 = mybir.dt.bfloat16
    EXP = mybir.ActivationFunctionType.Exp
    LOG = mybir.ActivationFunctionType.Ln

    nc = bass.Bass(trn_type="TRN2")

    x_d = nc.dram_tensor("x", [S, E], f32, kind="ExternalInput")
    wqt_d = nc.dram_tensor("wqt", [E, E], bf16, kind="ExternalInput")
    wkt_d = nc.dram_tensor("wkt", [E, E], bf16, kind="ExternalInput")
    bq_d = nc.dram_tensor("bq", [E, 1], f32, kind="ExternalInput")
    bk_d = nc.dram_tensor("bk", [E, 1], f32, kind="ExternalInput")
    wvt_d = nc.dram_tensor("wvt", [EA, VW], bf16, kind="ExternalInput")
    wot_d = nc.dram_tensor("wot", [EA, E], bf16, kind="ExternalInput")
    out_d = nc.dram_tensor("out", [S, E], f32, kind="ExternalOutput")

    from contextlib import ExitStack

    with tile.TileContext(nc) as tc, ExitStack() as ctx:
        singles = ctx.enter_context(tc.tile_pool(name="singles", bufs=1))

        ident = singles.tile([128, 128], f32)
        make_identity(nc, ident)

        ones_row = singles.tile([1, 1024], bf16)
        nc.vector.memset(ones_row, 1.0)

        # ---- weights / biases to SBUF ----
        def load_w(dram, width, rows):
            tiles = []
            nk = rows // 128
            for k in range(nk):
                t = singles.tile([128, width], bf16, tag=f"w{dram.name}{k}",
                                 name=f"w{dram.name}{k}")
                nc.sync.dma_start(out=t, in_=dram[k * 128:(k + 1) * 128, :])
                tiles.append(t)
            if rows % 128:
                t = singles.tile([1, width], bf16, tag=f"w{dram.name}b",
                                 name=f"w{dram.name}b")
                nc.sync.dma_start(out=t, in_=dram[E:EA, :])
                tiles.append(t)
            return tiles

        wq = load_w(wqt_d, E, E)
        wk = load_w(wkt_d, E, E)
        wv = load_w(wvt_d, VW, EA)
        wo = load_w(wot_d, E, EA)
        bqs, bks = [], []
        for m in range(NE):
            t = singles.tile([128, 1], f32, tag=f"bq{m}", name=f"bq{m}")
            nc.sync.dma_start(out=t, in_=bq_d[m * 128:(m + 1) * 128, :])
            bqs.append(t)
            t = singles.tile([128, 1], f32, tag=f"bk{m}", name=f"bk{m}")
            nc.sync.dma_start(out=t, in_=bk_d[m * 128:(m + 1) * 128, :])
            bks.append(t)

        # ---- P1: x -> xT (bf16) ----
        xt = [singles.tile([128, S], bf16, tag=f"xt{j}", name=f"xt{j}")
              for j in range(NE)]

        with tc.tile_pool(name="xload", bufs=8) as xload, \
             tc.tile_pool(name="ps_xt", bufs=4, space="PSUM") as ps_xt:
            xsb = []
            for i in range(NT):
                t = xload.tile([128, E], f32, tag="x", name=f"x{i}")
                nc.sync.dma_start(out=t, in_=x_d[i * 128:(i + 1) * 128, :])
                xsb.append(t)
            for ib in range(2):
                for j in range(NE):
                    ps = ps_xt.tile([128, 512], f32, tag="pxt",
                                    name=f"pxt{ib}_{j}")
                    for ii in range(4):
                        nc.tensor.transpose(
                            ps[:, ii * 128:(ii + 1) * 128],
                            xsb[ib * 4 + ii][:, j * 128:(j + 1) * 128],
                            ident,
                        )
                    nc.vector.tensor_copy(
                        xt[j][:, ib * 512:(ib + 1) * 512], ps)

        def xa(k):  # augmented xT rows
            return xt[k] if k < NE else ones_row

        # ---- P2a: V projection (augmented: bias row + ones cols) ----
        vt = [singles.tile([128, VW], bf16, tag=f"vt{i}", name=f"vt{i}")
              for i in range(NT)]
        with tc.tile_pool(name="ps_v", bufs=2, space="PSUM") as ps_v:
            for i in range(NT):
                ps = ps_v.tile([128, VW], f32, tag="pv", name=f"pv{i}")
                for off, sz in ((0, 512), (512, VW - 512)):
                    for k in range(NE + 1):
                        nc.tensor.matmul(
                            ps[:, off:off + sz],
                            lhsT=xa(k)[:, i * 128:(i + 1) * 128],
                            rhs=wv[k][:, off:off + sz],
                            start=(k == 0), stop=(k == NE),
                        )
                nc.vector.tensor_copy(vt[i], ps)

        # ---- P2b/P3 interleaved per head-pair ----
        qt = [singles.tile([128, S], bf16, tag=f"qt{j}", name=f"qt{j}")
              for j in range(NE)]
        kt = [singles.tile([128, S], bf16, tag=f"kt{j}", name=f"kt{j}")
              for j in range(NE)]
        catt = [singles.tile([128, S], bf16, tag=f"ct{j}", name=f"ct{j}")
                for j in range(NE)]

        with tc.tile_pool(name="exp", bufs=18) as expp, \
             tc.tile_pool(name="norm", bufs=4) as normp, \
             tc.tile_pool(name="ps_proj", bufs=2, space="PSUM") as ps_proj, \
             tc.tile_pool(name="ps_sc", bufs=2, space="PSUM") as ps_sc, \
             tc.tile_pool(name="ps_at", bufs=1, space="PSUM") as ps_at, \
             tc.tile_pool(name="dscr", bufs=4, space="DRAM") as dscr:
            for hp in range(H // 2):
                # Q/K projections for this pair (bias added on eviction)
                for dst, w, b in ((qt, wq, bqs), (kt, wk, bks)):
                    for sc in range(2):
                        ps = ps_proj.tile([128, 512], f32, tag="pp",
                                          name=f"pp{hp}_{sc}")
                        for k in range(NE):
                            nc.tensor.matmul(
                                ps,
                                lhsT=w[k][:, hp * 128:(hp + 1) * 128],
                                rhs=xt[k][:, sc * 512:(sc + 1) * 512],
                                start=(k == 0), stop=(k == NE - 1),
                            )
                        nc.vector.tensor_scalar_add(
                            dst[hp][:, sc * 512:(sc + 1) * 512], ps, b[hp])
                exps = [[], []]
                for t in range(NT):
                    for half in range(2):
                        lo, hi = half * 64, half * 64 + 64
                        ps = ps_sc.tile([128, 1024], f32, tag="sc",
                                        name=f"sc{hp}_{t}_{half}")
                        for sc in range(2):
                            nc.tensor.matmul(
                                ps[:, sc * 512:(sc + 1) * 512],
                                lhsT=kt[hp][lo:hi, t * 128:(t + 1) * 128],
                                rhs=qt[hp][lo:hi, sc * 512:(sc + 1) * 512],
                                start=True, stop=True,
                                tile_position=(lo, 0),
                            )
                        ex = expp.tile([128, 1024], bf16, tag="e",
                                       name=f"e{hp}_{t}_{half}")
                        nc.scalar.activation(ex, ps, EXP)
                        exps[half].append(ex)
                for half in range(2):
                    head = hp * 2 + half
                    pa = ps_at.tile([HW, 1024], f32, tag="at", name=f"at{head}")
                    for t in range(NT):
                        for sc in range(2):
                            nc.tensor.matmul(
                                pa[:, sc * 512:(sc + 1) * 512],
                                lhsT=vt[t][:, head * HW:(head + 1) * HW],
                                rhs=exps[half][t][:, sc * 512:(sc + 1) * 512],
                                start=(t == 0), stop=(t == NT - 1),
                            )
                    lg = normp.tile([1, 1024], f32, tag="lg", name=f"lg{head}")
                    nc.scalar.activation(lg, pa[64:65, :], LOG)
                    recip = normp.tile([1, 1024], f32, tag="r", name=f"r{head}")
                    nc.scalar.activation(recip, lg, EXP, scale=-1.0)
                    dsc = dscr.tile([1, 1024], f32, tag="d", name=f"d{head}")
                    nc.sync.dma_start(out=dsc, in_=recip)
                    recipb = normp.tile([64, 1024], f32, tag="rb",
                                        name=f"rb{head}")
                    nc.sync.dma_start(
                        out=recipb, in_=dsc[0].partition_broadcast(64))
                    nc.vector.tensor_mul(
                        catt[hp][half * 64:(half + 1) * 64, :],
                        pa[0:64, :], recipb)

        # ---- P4: output projection ----
        def ca(k):
            return catt[k] if k < NE else ones_row

        with tc.tile_pool(name="osb", bufs=3) as osb, \
             tc.tile_pool(name="ps_o", bufs=2, space="PSUM") as ps_o:
            for m in range(NT):
                ps = ps_o.tile([128, E], f32, tag="po", name=f"po{m}")
                for off, sz in ((0, 512), (512, E - 512)):
                    for k in range(NE + 1):
                        nc.tensor.matmul(
                            ps[:, off:off + sz],
                            lhsT=ca(k)[:, m * 128:(m + 1) * 128],
                            rhs=wo[k][:, off:off + sz],
                            start=(k == 0), stop=(k == NE),
                        )
                ot = osb.tile([128, E], f32, tag="o", name=f"ot{m}")
                nc.vector.tensor_copy(ot, ps)
                nc.sync.dma_start(out=out_d[m * 128:(m + 1) * 128, :], in_=ot)

    if split_waits:
        _split_multiwaits(nc)
    return nc


def _prep_weights(Wq, bq, Wk, bk, Wv, bv, Wo, bo):
    bf16 = ml_dtypes.bfloat16
    scale = 1.0 / np.sqrt(np.float32(DH))

    wqt = (np.asarray(Wq, np.float32).reshape(H * DH, E) * scale).T.astype(bf16)
    wkt = np.asarray(Wk, np.float32).reshape(H * DH, E).T.astype(bf16)
    bqv = (np.asarray(bq, np.float32).reshape(E, 1) * scale).astype(np.float32)
    bkv = np.asarray(bk, np.float32).reshape(E, 1).astype(np.float32)

    wvt = np.zeros((EA, VW), np.float32)
    Wv = np.asarray(Wv, np.float32)
    bv = np.asarray(bv, np.float32)
    for h in range(H):
        wvt[0:E, h * HW:h * HW + DH] = Wv[h].T
        wvt[E, h * HW:h * HW + DH] = bv[h]
        wvt[E, h * HW + DH] = 1.0
    wvt = wvt.astype(bf16)

    Wo = np.asarray(Wo, np.float32)
    bo = np.asarray(bo, np.float32)
    wot = np.concatenate([Wo.T, bo.reshape(1, E)], axis=0).astype(bf16)
    return wqt, wkt, bqv, bkv, wvt, wot


def _install_ntff_shim():
    """Provide antenv.axon_hooks (absent in this image) so trace=True can
    drive NRT profiling through libaxon_pjrt.so.  Dev-only; harmless no-op
    when anything is missing."""
    import sys, types
    try:
        import antenv.axon_hooks  # noqa
        return
    except ImportError:
        pass
    try:
        import antenv
        mod = types.ModuleType("antenv.axon_hooks")
        _state = {}
        mod.set_axon_ntff_profile_hook = lambda h: _state.update(h=h)
        mod.get_axon_ntff_profile_hook = lambda: _state.get("h")
        sys.modules["antenv.axon_hooks"] = mod
        antenv.axon_hooks = mod
        from trn_agent_boot.trn_boot import _ntff_profile_via_ctypes
        hook = _ntff_profile_via_ctypes("/opt/axon/libaxon_pjrt.so")
        if hook is not None:
            mod.set_axon_ntff_profile_hook(hook)
    except Exception as e:  # pragma: no cover
        print(f"ntff shim failed: {e}")


def kernel(x, Wq, bq, Wk, bk, Wv, bv, Wo, bo):
    from concourse.bass_utils import run_bass_kernel_spmd

    if "nc" not in _cache:
        _cache["nc"] = _build_bass()
    nc = _cache["nc"]

    wqt, wkt, bqv, bkv, wvt, wot = _prep_weights(Wq, bq, Wk, bk, Wv, bv, Wo, bo)
    x = np.asarray(x, np.float32)
    in_maps = [
        {"x": np.ascontiguousarray(x[b]),
         "wqt": wqt, "wkt": wkt, "bq": bqv, "bk": bkv,
         "wvt": wvt, "wot": wot}
        for b in range(B)
    ]
    trace = bool(int(os.environ.get("MHA_TRACE", "0")))
    if trace:
        _install_ntff_shim()
    res = run_bass_kernel_spmd(nc, in_maps, list(range(B)), trace=trace)
    _cache["last_results"] = res
    return np.stack([res.results[b]["out"] for b in range(B)]).astype(np.float32)
